# revision 30
# baseline (speedup 1.0000x reference)
"""AceStep GQA attention block on 8 TRN2 NeuronCores.

Sharding: tensor-parallel over heads (TP=2, kv heads stay grouped with
their q heads) x data-parallel over batch (DP=4).  Core i handles batch
b = i // 2 and head group g = i % 2 (q heads 8g..8g+7, kv heads 2g,2g+1).
Each core computes a partial output projection (its head group's slice of
Wo rows); the host sums the two partials per batch.

Device-side dataflow per core (all matmuls f32r = full-rate fp32):
  pass p in {0,1}:  (kv head p, q heads 4p..4p+3)
    proj:  xT tiles (stationary) x Wq/Wk/Wv slices -> Q/K/V token-major,
           per-head RMSNorm via ACT square+accum, rsqrt; RoPE fused with the
           norm scale via scalar_tensor_tensor (cos/sin tables carry the
           norm weights, folded on host); PE-transpose Q,K to [d, t].
    attn:  ST[sk,sq] = KT_tile.T @ QT chunk; E = exp(SCALE*ST) on ACT;
           denominator = ones.T @ E (PE, accumulated over sk tiles);
           OUT_T[d,sq] = V_tile.T @ E accumulated over sk tiles;
           A = OUT_T * bcast(1/denom)  (bcast via K=1 matmul).
  final: out[t,:] += A_h[:,t].T @ Wo_h rows, accumulated over 8 heads.
"""

import sys

if "/opt/trn_rl_repo" not in sys.path:
    sys.path.insert(0, "/opt/trn_rl_repo")

from contextlib import ExitStack

import numpy as np
import ml_dtypes

import concourse.bass as bass
import concourse.mybir as mybir
import concourse.tile as tile
from concourse.bass import ts, ds
from concourse.masks import make_identity
from concourse.vector_clock import ScopedClock, VectorClock
from concourse.bass_utils import run_bass_kernel_spmd

B, S, HID = 4, 2048, 2048
H, KV, D = 16, 4, 128
EPS = 1e-6
SCALE = float(D) ** -0.5
NCORES = 8
TP = 2
QH = H // TP            # 8 q heads per core
KVH = KV // TP          # 2 kv heads per core = passes
QHP = QH // KVH         # 4 q heads per pass
NT = S // 128           # 16 token tiles
NHID = HID // 128       # 16 hid tiles
CH = 512                # sq chunk width
NCH = S // CH           # 4 chunks
F32 = mybir.dt.float32
F32R = mybir.dt.float32r
BF16 = mybir.dt.bfloat16
CH2 = 1024              # paired sq chunk width (2 PSUM banks)
MULT = mybir.AluOpType.mult
AF = mybir.ActivationFunctionType


def _patched_drain_and_barrier(self, tick_clock, wait_clock):
    # Walrus CoreV3 rejects >1-2 sem waits on a CTRL (Drain) instruction.
    # Split the final global-clock wait into one single-wait drain per proc.
    gc = tick_clock.global_clock
    n = len(gc)
    for p in range(n):
        t = gc[p]
        if t > 0:
            vec = [0] * n
            vec[p] = t
            d = self.nc.sync.drain()
            wait_clock.add_sem_waits(d.ins, ScopedClock({None: VectorClock(vec)}))
    self.nc.sync.drain()
    self.nc.all_engine_barrier()
    assert self.sems is not None
    popped = self.nc._tile_sem_poison_stack.pop()
    assert popped is self._sem_poison
    self.nc.clear_and_free_semaphores(list(self.sems.allocated().values()))
    self.nc.all_engine_barrier()


tile.TileContext._drain_and_barrier = _patched_drain_and_barrier

def _max_waits(inst):
    # Walrus CoreV2/V3 setupSyncWait takes a single wait per TPB instruction;
    # EventSemaphore can hold two.
    if isinstance(inst, mybir.InstEventSemaphore):
        return 2
    return 1


def _legalize_waits(nc):
    """Walrus CoreV3 rejects instructions carrying too many sync waits.
    Spill the excess onto no-op carrier instructions inserted just before,
    on the same engine stream."""
    n_new = 0
    for f in nc.m.functions:
        for bb in f.blocks:
            insts = bb.instructions
            out = []
            changed = False
            for inst in insts:
                si = getattr(inst, "sync_info", None)
                waits = list(si.on_wait) if (si and si.on_wait) else []
                mw = _max_waits(inst)
                if len(waits) > mw:
                    spill, keep = waits[:-mw], waits[-mw:]
                    for i in range(0, len(spill)):
                        nop = mybir.InstNoOp(
                            name=f"waitspill-{n_new}",
                            engine=inst.engine,
                            sync_info=mybir.SyncInfo(
                                on_wait=spill[i : i + 1], on_update=[]
                            ),
                            bass_nofuse=True,
                        )
                        n_new += 1
                        out.append(nop)
                    si.on_wait = keep
                    changed = True
                out.append(inst)
            if changed:
                bb.instructions = out
    return n_new


def _emit(nc, tc, io, phases=("proj", "attn", "oproj")):
    xT, wq, wkv, rope4, wo, ones_d, ones_bf, out = io

    xT = xT.rearrange("(j p) t -> p j t", p=128)       # [128, NHID, S]
    wq = wq.rearrange("(j p) n -> p j n", p=128)       # [128, NHID, QH*D]
    wkv = wkv.rearrange("(j p) a n -> p j a n", p=128)  # [128, NHID, KVH, 256]
    wo_r = wo.rearrange("(h p) n -> p h n", p=128)     # [128, QH, HID]

    with ExitStack() as top:
        const = top.enter_context(tc.tile_pool(name="const", bufs=1))
        ident = const.tile([128, 128], F32)
        make_identity(nc, ident)
        ones_col = const.tile([128, 1], BF16)
        nc.sync.dma_start(out=ones_col, in_=ones_bf[:, 0:1])
        ones_row = const.tile([1, 128], F32R)
        nc.sync.dma_start(out=ones_row, in_=ones_d[0:1, :])
        eps_t = const.tile([128, 1], F32)
        nc.vector.memset(eps_t, EPS)
        # pin the ACT table set to natural_log_exp_and_others (has exp, ln,
        # square, copy) so no table switches happen mid-kernel
        dummy = const.tile([128, 1], F32)
        nc.scalar.activation(dummy, eps_t, AF.Ln)

        qa_pool = top.enter_context(tc.tile_pool(name="qa", bufs=1))
        QA = qa_pool.tile([128, QH, S], BF16)           # QT, later A, [d, h, t]
        kt_pool = top.enter_context(tc.tile_pool(name="kt", bufs=1))
        KT = kt_pool.tile([128, S], BF16)               # per-pass KT [d, t]
        v_pool = top.enter_context(tc.tile_pool(name="v", bufs=1))
        VT = v_pool.tile([128, NT, D], BF16)            # per-pass V [t%128, tt, d]

        epool = top.enter_context(tc.tile_pool(name="e", bufs=8))
        wq_pool = top.enter_context(tc.tile_pool(name="wq", bufs=2))

        def load_weights(p):
            wq_sb = wq_pool.tile([128, NHID, QHP * D], BF16, tag="wq", name="wq_sb")
            wkv_sb = wq_pool.tile([128, NHID, 256], BF16, tag="wkv", name="wkv_sb")
            for jq in range(4):
                nc.scalar.dma_start(
                    out=wq_sb[:, ds(jq * 4, 4), :],
                    in_=wq[:, ds(jq * 4, 4), ds(p * QHP * D, QHP * D)],
                )
            nc.scalar.dma_start(out=wkv_sb, in_=wkv[:, :, p, :])
            return wq_sb, wkv_sb

        cur_w = load_weights(0) if "proj" in phases else None
        small = top.enter_context(tc.tile_pool(name="small", bufs=4))

        wo_sb = None

        for p in range(KVH):
            # ---------------- projection phase (pass p) ----------------
            with ExitStack() as ph:
                if "proj" in phases:
                    wq_sb, wkv_sb = cur_w
                    xpool = ph.enter_context(tc.tile_pool(name="x", bufs=4))
                    rpool = ph.enter_context(tc.tile_pool(name="rope", bufs=2))
                    spool = ph.enter_context(tc.tile_pool(name="scr", bufs=4))
                    qrpool = ph.enter_context(tc.tile_pool(name="qr", bufs=10))
                    psq = ph.enter_context(tc.tile_pool(name="psq", bufs=3, space="PSUM"))
                    pskv = ph.enter_context(tc.tile_pool(name="pskv", bufs=2, space="PSUM"))
                    pst_pool = ph.enter_context(
                        tc.tile_pool(name="pst", bufs=3, space="PSUM")
                    )

                    # transpose+copy of tile tt is deferred until after tile
                    # tt+1's projection matmuls so the PE never waits on the
                    # ACT/DVE norm+rope chain.
                    pending = []
                    new_pending = []

                    def flush_pending():
                        for qr_t, dst in pending:
                            psT = pst_pool.tile([128, 128], F32)
                            nc.tensor.transpose(psT, qr_t, ident)
                            nc.scalar.copy(dst, psT)
                        pending.clear()

                    for tt in range(NT):
                        xx = xpool.tile([128, NHID, 128], BF16, tag="xx")
                        nc.sync.dma_start(out=xx, in_=xT[:, :, ts(tt, 128)])
                        rp = rpool.tile([128, 4, 128], F32, tag="rp")
                        nc.sync.dma_start(out=rp, in_=rope4[ts(tt, 128), :, :])
                        cwq_t = rp[:, 0, :]
                        swq_t = rp[:, 1, :]
                        cwk_t = rp[:, 2, :]
                        swk_t = rp[:, 3, :]

                        psQ = psq.tile([128, QHP * D], F32)
                        psKV = pskv.tile([128, 256], F32)
                        for j in range(NHID):
                            nc.tensor.matmul(
                                psQ,
                                xx[:, j, :],
                                wq_sb[:, j, :],
                                start=(j == 0),
                                stop=(j == NHID - 1),
                            )
                        for j in range(NHID):
                            nc.tensor.matmul(
                                psKV,
                                xx[:, j, :],
                                wkv_sb[:, j, :],
                                start=(j == 0),
                                stop=(j == NHID - 1),
                            )

                        # batched RMSNorm scale: 5 squares (4 Q heads + K)
                        # accumulate into one [128,5]; one ln + one exp.
                        scratch = spool.tile([128, 128], F32, tag="scr")
                        ssq5 = small.tile([128, 5], F32, tag="ssq")
                        s15 = small.tile([128, 5], F32, tag="s1")
                        r5 = small.tile([128, 5], F32, tag="r")
                        for jh in range(QHP):
                            nc.scalar.activation(
                                scratch,
                                psQ[:, ts(jh, 128)],
                                AF.Square,
                                accum_out=ssq5[:, jh : jh + 1],
                            )
                        nc.scalar.activation(
                            scratch,
                            psKV[:, 0:128],
                            AF.Square,
                            accum_out=ssq5[:, 4:5],
                        )
                        nc.scalar.activation(s15, ssq5, AF.Ln, bias=eps_t, scale=1.0 / D)
                        nc.scalar.activation(r5, s15, AF.Exp, scale=-0.5)

                        def norm_rope(src, cw_t, sw_t, r, dst):
                            m1 = spool.tile([128, 128], F32, tag="m1")
                            m2 = spool.tile([128, 128], F32, tag="m2")
                            qr = qrpool.tile([128, 128], F32, tag="qr")
                            nc.vector.scalar_tensor_tensor(
                                out=m1, in0=src, scalar=r, in1=cw_t, op0=MULT, op1=MULT
                            )
                            nc.vector.scalar_tensor_tensor(
                                out=m2[:, 0:64],
                                in0=src[:, 64:128],
                                scalar=r,
                                in1=sw_t[:, 0:64],
                                op0=MULT,
                                op1=MULT,
                            )
                            nc.vector.scalar_tensor_tensor(
                                out=m2[:, 64:128],
                                in0=src[:, 0:64],
                                scalar=r,
                                in1=sw_t[:, 64:128],
                                op0=MULT,
                                op1=MULT,
                            )
                            nc.vector.tensor_add(qr, m1, m2)
                            new_pending.append((qr, dst))

                        for jh in range(QHP):
                            hl = p * QHP + jh
                            norm_rope(
                                psQ[:, ts(jh, 128)],
                                cwq_t,
                                swq_t,
                                r5[:, jh : jh + 1],
                                QA[:, hl, ts(tt, 128)],
                            )
                        norm_rope(
                            psKV[:, 0:128],
                            cwk_t,
                            swk_t,
                            r5[:, 4:5],
                            KT[:, ts(tt, 128)],
                        )
                        nc.scalar.copy(VT[:, tt, :], psKV[:, 128:256])
                        flush_pending()
                        pending.extend(new_pending)
                        new_pending.clear()
                    flush_pending()
                    if p + 1 < KVH:
                        cur_w = load_weights(p + 1)

            # load Wo after the last projection phase frees its pools
            if p == KVH - 1 and "oproj" in phases:
                wo_pool = top.enter_context(tc.tile_pool(name="wo", bufs=1))
                wo_sb = wo_pool.tile([128, QH, HID], BF16)
                nc.sync.dma_start(out=wo_sb, in_=wo_r)

            # ---------------- attention phase (pass p) ----------------
            if "attn" not in phases:
                continue
            # Processed in sq chunk-pairs of 1024: scores fill a 2-bank PSUM
            # tile, one wide exp per sk tile (amortizes ACT per-op overhead),
            # denominator 2-way column-tiled on the PE (concurrent groups).
            with ExitStack() as ph:
                accpool = ph.enter_context(tc.tile_pool(name="acc", bufs=1))
                pss = ph.enter_context(tc.tile_pool(name="pss", bufs=2, space="PSUM"))
                pso = ph.enter_context(tc.tile_pool(name="pso", bufs=2, space="PSUM"))

                # The normalization tail of chunk-pair K (denominator
                # colsum, reciprocal, broadcast, final multiply) is deferred
                # into chunk-pair K+1's stream so the PE never waits on the
                # DVE/Pool partial-sum chains.
                tail_prev = [None]

                def make_tail(hl, cp, psO2, accs):
                    def tail():
                        psD = [
                            pss.tile([1, CH], F32, tag="s", name=f"psD{_h}")
                            for _h in range(2)
                        ]
                        for h2 in range(2):
                            for gi in range(4):
                                nc.tensor.matmul(
                                    psD[h2],
                                    ones_col,
                                    accs[gi][:, ds(h2 * CH, CH)],
                                    start=(gi == 0),
                                    stop=(gi == 3),
                                )
                        for h2 in range(2):
                            c = cp * 2 + h2
                            rd = small.tile([1, CH], F32R, tag="rd")
                            with nc.allow_low_precision(reason="f32r bcast rhs"):
                                nc.vector.reciprocal(rd, psD[h2])
                            psB = pss.tile([128, CH], F32, tag="s")
                            nc.tensor.matmul(
                                psB, ones_row, rd, start=True, stop=True
                            )
                            bc = epool.tile([128, CH], F32, tag="bc")
                            nc.vector.tensor_copy(bc, psB)
                            nc.vector.tensor_mul(
                                QA[:, hl, ds(c * CH, CH)],
                                psO2[:, ds(h2 * CH, CH)],
                                bc,
                            )
                    return tail

                for jh in range(QHP):
                    hl = p * QHP + jh
                    for cp in range(S // CH2):
                        etiles = [None] * NT

                        def scores(i):
                            psS = pss.tile([128, CH2], F32, tag="s")
                            for h2 in range(2):
                                nc.tensor.matmul(
                                    psS[:, ds(h2 * CH, CH)],
                                    KT[:, ts(i, 128)],
                                    QA[:, hl, ds(cp * CH2 + h2 * CH, CH)],
                                    start=True,
                                    stop=True,
                                )
                            e = epool.tile([128, CH2], BF16, tag="e")
                            nc.scalar.activation(e, psS, AF.Exp, scale=SCALE)
                            etiles[i] = e

                        psO2 = pso.tile([128, CH2], F32, tag="o", name="psO2")
                        scores(0)
                        scores(1)
                        accs = None
                        for i in range(NT):
                            if i == 0 and tail_prev[0] is not None:
                                tail_prev[0]()
                            if i == 0:
                                # allocated after the previous tail so WAR
                                # deps resolve in emission order
                                accs = [
                                    accpool.tile([128, CH2], BF16, tag=f"acc{_c}", name=f"acc{_c}")
                                    for _c in range(4)
                                ]
                            e = etiles[i]
                            for h2 in range(2):
                                eh = e[:, ds(h2 * CH, CH)]
                                nc.tensor.matmul(
                                    psO2[:, ds(h2 * CH, CH)],
                                    VT[:, i, :],
                                    eh,
                                    start=(i == 0),
                                    stop=(i == NT - 1),
                                )
                            if i + 2 < NT:
                                scores(i + 2)
                            # softmax denominator: 4 stride-4 partial-sum
                            # chains, 2 on DVE + 2 on Pool; short colsum
                            # matmuls happen in the deferred tail.
                            g = i % 4
                            eng = nc.vector
                            if 4 <= i < 8:
                                eng.tensor_add(accs[g], etiles[i - 4], e)
                            elif i >= 8:
                                eng.tensor_add(accs[g], accs[g], e)
                        tail_prev[0] = make_tail(hl, cp, psO2, accs)
                if tail_prev[0] is not None:
                    tail_prev[0]()

        # ---------------- output projection ----------------
        if "oproj" not in phases:
            return
        with ExitStack() as ph:
            psc = ph.enter_context(tc.tile_pool(name="psc", bufs=6, space="PSUM"))
            opool = ph.enter_context(tc.tile_pool(name="osb", bufs=3))
            for tt in range(NT):
                osb = opool.tile([128, NCH, CH], F32, tag="osb")
                for nch in range(NCH):
                    psC = psc.tile([128, CH], F32)
                    for h in range(QH):
                        nc.tensor.matmul(
                            psC,
                            QA[:, h, ts(tt, 128)],
                            wo_sb[:, h, ds(nch * CH, CH)],
                            start=(h == 0),
                            stop=(h == QH - 1),
                        )
                    if nch % 2 == 0:
                        nc.scalar.copy(osb[:, nch, :], psC)
                    else:
                        nc.vector.tensor_copy(osb[:, nch, :], psC)
                nc.sync.dma_start(
                    out=out[ts(tt, 128), :].rearrange("p (a c) -> p a c", a=NCH),
                    in_=osb,
                )


_PROGRAM = None


def _build_program(legalize=True, bodies=1, phases=("proj", "attn", "oproj")):
    global _PROGRAM
    if _PROGRAM is not None and legalize and bodies == 1 and len(phases) == 3:
        return _PROGRAM
    nc = bass.Bass("TRN2", target_bir_lowering=False, debug=False, num_devices=NCORES)
    xT = nc.dram_tensor("xT", [HID, S], BF16, kind="ExternalInput").ap()
    wq = nc.dram_tensor("wq", [HID, QH * D], BF16, kind="ExternalInput").ap()
    wkv = nc.dram_tensor("wkv", [HID, KVH, 256], BF16, kind="ExternalInput").ap()
    rope4 = nc.dram_tensor("rope4", [S, 4, D], F32, kind="ExternalInput").ap()
    wo = nc.dram_tensor("wo", [QH * D, HID], BF16, kind="ExternalInput").ap()
    ones_d = nc.dram_tensor("ones", [128, 128], F32R, kind="ExternalInput").ap()
    ones_bf = nc.dram_tensor("ones_bf", [128, 2], BF16, kind="ExternalInput").ap()
    out = nc.dram_tensor("out", [S, HID], F32, kind="ExternalOutput").ap()
    with tile.TileContext(nc) as tc:
        for _rep in range(bodies):
            _emit(nc, tc, (xT, wq, wkv, rope4, wo, ones_d, ones_bf, out), phases=phases)
    if legalize:
        _legalize_waits(nc)
        if bodies == 1 and len(phases) == 3:
            _PROGRAM = nc
    return nc


def _host_prep(hidden_states, cos, sin, Wq, Wk, Wv, Wo, q_norm_w, k_norm_w):
    """Build per-core input maps."""
    f = np.float32
    cos = np.asarray(cos, f)
    sin = np.asarray(sin, f)
    qw = np.asarray(q_norm_w, f)
    kw = np.asarray(k_norm_w, f)

    def fold(w):
        cw = (cos * w[None, :]).astype(f)
        sw = np.empty_like(sin)
        half = D // 2
        sw[:, :half] = -sin[:, :half] * w[None, half:]
        sw[:, half:] = sin[:, half:] * w[None, :half]
        return np.ascontiguousarray(cw), np.ascontiguousarray(sw)

    cwq, swq = fold(qw)
    cwk, swk = fold(kw)
    rope4 = np.stack([cwq, swq, cwk, swk], axis=1)  # [S, 4, D]

    Wq = np.asarray(Wq, f)
    Wk = np.asarray(Wk, f)
    Wv = np.asarray(Wv, f)
    Wo = np.asarray(Wo, f)
    hs = np.asarray(hidden_states, f)

    bf = ml_dtypes.bfloat16
    in_maps = []
    for i in range(NCORES):
        b, g = i // TP, i % TP
        xT = np.ascontiguousarray(hs[b].T).astype(bf)           # [HID, S]
        wq_g = np.ascontiguousarray(Wq[:, g * QH * D:(g + 1) * QH * D]).astype(bf)
        wkv = np.empty((HID, KVH, 256), f)
        for p in range(KVH):
            kvh = g * KVH + p
            wkv[:, p, 0:128] = Wk[:, kvh * D:(kvh + 1) * D]
            wkv[:, p, 128:256] = Wv[:, kvh * D:(kvh + 1) * D]
        wkv = wkv.astype(bf)
        wo_g = np.ascontiguousarray(Wo[g * QH * D:(g + 1) * QH * D, :]).astype(bf)
        in_maps.append(
            {
                "xT": xT,
                "wq": wq_g,
                "wkv": wkv,
                "rope4": rope4,
                "wo": wo_g,
                "ones": np.ones((128, 128), f),
                "ones_bf": np.ones((128, 2), ml_dtypes.bfloat16),
            }
        )
    return in_maps


def run_cores(in_maps, trace=False, **kwargs):
    nc = _build_program()
    return run_bass_kernel_spmd(
        nc, in_maps, core_ids=list(range(NCORES)), trace=trace, **kwargs
    )


def kernel(hidden_states, cos, sin, Wq, Wk, Wv, Wo, q_norm_w, k_norm_w):
    in_maps = _host_prep(
        hidden_states, cos, sin, Wq, Wk, Wv, Wo, q_norm_w, k_norm_w
    )
    res = run_cores(in_maps, trace=False)
    out = np.empty((B, S, HID), np.float32)
    for b in range(B):
        out[b] = res.results[b * TP]["out"]
        out[b] += res.results[b * TP + 1]["out"]
    return out

